# revision 6
# baseline (speedup 1.0000x reference)
"""AutoFocalLoss regression kernel for Trainium2, 8-core data-parallel.

Reference computation (all fp32):
    d      = |pred - target|                          (16,777,216 elements)
    mean_d = mean(d)
    var    = sum((d - mean_d)^2) / (n - 1)
    p      = mean(1 - erf((d / var) * 1/sqrt(2)))
    gamma  = -log(p)
    loss   = mean(d * (1-p)^gamma + log(var + 1))
           = mean_d * (1-p)^gamma + log(var + 1)      (elementwise part is affine in d)

So the device only has to produce three global sums: sum(d), sum(d^2) and
sum(erf(s*d)) with s = 1/(sqrt(2)*var).  The erf pass needs the *global* var,
so the kernel does: per-core partial sums -> 1KB AllReduce -> on-device var /
erf-scale -> erf pass over the SBUF-resident |diff| buffer -> per-core erf
partial sums.  Final O(1) scalar math happens on the host.

Per core: 2,097,152 elements (8 MB) viewed as [128 partitions x 16384].
"""

import numpy as np

P = 128
N_CORES = 8
ROWS, COLS = 4194304, 4
N_TOTAL = ROWS * COLS                    # 16,777,216
PER_CORE = N_TOTAL // N_CORES            # 2,097,152
FREE = PER_CORE // P                     # 16,384
F_TILE = 2048
INV_SQRT2 = 0.7071067811865476

_CACHE = {}


def _build(free=FREE, n_total=float(N_TOTAL), f_tile=F_TILE, n_cores=N_CORES,
           act_name="Erf"):
    import concourse.mybir as mybir
    import concourse.tile as tile
    from concourse import bass_isa
    from concourse.bacc import Bacc

    f32 = mybir.dt.float32
    AF = mybir.ActivationFunctionType
    ALU = mybir.AluOpType
    X = mybir.AxisListType.X

    T = free // f_tile
    nc = Bacc()
    pred = nc.dram_tensor("pred", [P, free], f32, kind="ExternalInput")
    targ = nc.dram_tensor("target", [P, free], f32, kind="ExternalInput")
    out = nc.dram_tensor("out", [P, 3], f32, kind="ExternalOutput")

    with tile.TileContext(nc) as tc:
        with (
            tc.tile_pool(name="io", bufs=3) as io_pool,
            tc.tile_pool(name="work", bufs=2) as work_pool,
            tc.tile_pool(name="persist", bufs=1) as persist,
            tc.tile_pool(name="dram", bufs=1, space="DRAM") as dram_pool,
        ):
            dbuf = persist.tile([P, free], f32, name="dbuf")
            sumd_cols = persist.tile([P, T], f32, name="sumd_cols")
            sumsq_cols = persist.tile([P, T], f32, name="sumsq_cols")
            erf_cols = persist.tile([P, T], f32, name="erf_cols")

            # Phase A: stream tiles, d = |pred-target| stays in SBUF,
            # accumulate per-partition sum(d) (ACT) and sum(d^2) (DVE).
            for t in range(T):
                sl = slice(t * f_tile, (t + 1) * f_tile)
                pt = io_pool.tile([P, f_tile], f32, name="pt", tag="pt")
                tt = io_pool.tile([P, f_tile], f32, name="tt", tag="tt")
                nc.sync.dma_start(out=pt[:], in_=pred[:, sl])
                nc.sync.dma_start(out=tt[:], in_=targ[:, sl])
                df = work_pool.tile([P, f_tile], f32, name="df", tag="df")
                nc.vector.tensor_sub(df[:], pt[:], tt[:])
                nc.scalar.activation(
                    dbuf[:, sl], df[:], AF.Abs,
                    accum_out=sumd_cols[:, t : t + 1],
                )
                junk = work_pool.tile([P, f_tile], f32, name="junk", tag="junk")
                nc.scalar.activation(
                    junk[:], df[:], AF.Square,
                    accum_out=sumsq_cols[:, t : t + 1],
                )

            stats = persist.tile([P, 2], f32, name="stats")
            nc.vector.reduce_sum(stats[:, 0:1], sumd_cols[:], axis=X)
            nc.vector.reduce_sum(stats[:, 1:2], sumsq_cols[:], axis=X)

            # Cross-core AllReduce of the [128,2] per-partition partials.
            cc_in = dram_pool.tile([P, 2], f32, name="cc_in")
            cc_out = dram_pool.tile([P, 2], f32, name="cc_out",
                                    addr_space="Shared")
            nc.sync.dma_start(out=cc_in[:], in_=stats[:])
            nc.gpsimd.collective_compute(
                "AllReduce", ALU.add,
                replica_groups=[list(range(n_cores))],
                ins=[cc_in.opt()], outs=[cc_out.opt()],
            )
            tot_pp = persist.tile([P, 2], f32, name="tot_pp")
            nc.sync.dma_start(out=tot_pp[:], in_=cc_out[:])
            tot = persist.tile([P, 2], f32, name="tot")
            nc.gpsimd.partition_all_reduce(
                tot[:], tot_pp[:], channels=P, reduce_op=bass_isa.ReduceOp.add
            )

            # var = (sum_sq - sum_d * mean) / (n-1);  s = INV_SQRT2 / var
            # computed redundantly on every partition ([P,1] lanes).
            mean = persist.tile([P, 1], f32, name="mean")
            nc.vector.tensor_scalar_mul(mean[:], tot[:, 0:1], 1.0 / n_total)
            t1 = persist.tile([P, 1], f32, name="t1")
            nc.vector.tensor_mul(t1[:], mean[:], tot[:, 0:1])
            t2 = persist.tile([P, 1], f32, name="t2")
            nc.vector.tensor_sub(t2[:], tot[:, 1:2], t1[:])
            var = persist.tile([P, 1], f32, name="var")
            nc.vector.tensor_scalar_mul(var[:], t2[:], 1.0 / (n_total - 1.0))
            rvar = persist.tile([P, 1], f32, name="rvar")
            nc.vector.reciprocal(rvar[:], var[:])
            svec = persist.tile([P, 1], f32, name="svec")
            nc.vector.tensor_scalar_mul(svec[:], rvar[:], INV_SQRT2)

            # Phase B: erf(s*d) over the resident d buffer; d >= 0 so the
            # ACT accumulator sum is already the sum of erf values.
            for t in range(T):
                sl = slice(t * f_tile, (t + 1) * f_tile)
                esc = work_pool.tile([P, f_tile], f32, name="esc", tag="junk")
                nc.scalar.activation(
                    esc[:], dbuf[:, sl], getattr(AF, act_name),
                    scale=svec[:, 0:1],
                    accum_out=erf_cols[:, t : t + 1],
                )

            outsb = persist.tile([P, 3], f32, name="outsb")
            nc.vector.reduce_sum(outsb[:, 0:1], erf_cols[:], axis=X)
            nc.vector.tensor_copy(outsb[:, 1:3], tot[:])
            nc.sync.dma_start(out=out[:, :], in_=outsb[:])

    nc.finalize()
    return nc


def _get_nc():
    if "nc" not in _CACHE:
        _CACHE["nc"] = _build()
    return _CACHE["nc"]


def _finish(results):
    """Host-side O(1) scalar math from per-core device sums."""
    r0 = np.asarray(results[0]["out"], dtype=np.float64)
    sum_d = r0[0, 1]
    sum_sq = r0[0, 2]
    s_erf = 0.0
    for r in results:
        s_erf += float(np.asarray(r["out"])[:, 0].sum(dtype=np.float64))
    n = float(N_TOTAL)
    mean_d = sum_d / n
    var = (sum_sq - sum_d * mean_d) / (n - 1.0)
    p = 1.0 - s_erf / n
    gamma = -np.log(p)
    loss = mean_d * (1.0 - p) ** gamma + np.log1p(var)
    return np.array(loss, dtype=np.float32)


def kernel(pred: np.ndarray, target: np.ndarray) -> np.ndarray:
    from concourse.bass_utils import run_bass_kernel_spmd

    nc = _get_nc()
    p = np.ascontiguousarray(pred, dtype=np.float32).reshape(-1)
    t = np.ascontiguousarray(target, dtype=np.float32).reshape(-1)
    in_maps = []
    for c in range(N_CORES):
        sl = slice(c * PER_CORE, (c + 1) * PER_CORE)
        in_maps.append({
            "pred": p[sl].reshape(P, FREE),
            "target": t[sl].reshape(P, FREE),
        })
    res = run_bass_kernel_spmd(nc, in_maps, list(range(N_CORES)))
    return _finish(res.results)


# revision 7
# speedup vs baseline: 1.2544x; 1.2544x over previous
"""AutoFocalLoss regression kernel for Trainium2, 8-core data-parallel.

Reference computation (all fp32):
    d      = |pred - target|                          (16,777,216 elements)
    mean_d = mean(d)
    var    = sum((d - mean_d)^2) / (n - 1)
    p      = mean(1 - erf((d / var) * 1/sqrt(2)))
    gamma  = -log(p)
    loss   = mean(d * (1-p)^gamma + log(var + 1))
           = mean_d * (1-p)^gamma + log(var + 1)      (elementwise part is affine in d)

Device produces three global sums: sum(d), sum(d^2), sum(erf(s*d)) with
s = 1/(sqrt(2)*var).  The erf pass needs the *global* var, so per-core
partial sums are exchanged mid-kernel with an 8-byte-per-rank AllGather
(cheaper than AllReduce: ~5us vs ~36us measured), summed on-device, and the
erf scale is broadcast to all partitions.  |diff| stays resident in SBUF
(8 MB/core) so the data is only read from HBM once.  Final O(1) scalar math
happens on the host.

Layout per core: 2,097,152 elements viewed as [128 partitions x 16384].
Phase A streams 16 x 1MB DMA tiles (one per HW DMA engine).  The erf pass
runs in-place over the resident buffer in 4096-wide chunks.  A dummy Erf at
kernel start pins the 'sigmoid_and_others' ACT table set, which contains
Abs, Square and Erf - so only one table load total.
"""

import numpy as np

P = 128
N_CORES = 8
ROWS, COLS = 4194304, 4
N_TOTAL = ROWS * COLS                    # 16,777,216
PER_CORE = N_TOTAL // N_CORES            # 2,097,152
FREE = PER_CORE // P                     # 16,384
F_TILE = 2048
INV_SQRT2 = 0.7071067811865476

_CACHE = {}


def _build(free=FREE, n_total=float(N_TOTAL), f_tile=F_TILE, n_cores=N_CORES,
           act_name="Erf", f_erf=None):
    import concourse.mybir as mybir
    import concourse.tile as tile
    from concourse import bass_isa
    from concourse.bacc import Bacc

    f32 = mybir.dt.float32
    AF = mybir.ActivationFunctionType
    ALU = mybir.AluOpType
    X = mybir.AxisListType.X
    act_fn = getattr(AF, act_name)

    T = free // f_tile
    if f_erf is None:
        f_erf = min(free, 2 * f_tile)
    TE = free // f_erf
    nc = Bacc()
    pred = nc.dram_tensor("pred", [P, free], f32, kind="ExternalInput")
    targ = nc.dram_tensor("target", [P, free], f32, kind="ExternalInput")
    out = nc.dram_tensor("out", [P, 3], f32, kind="ExternalOutput")

    with tile.TileContext(nc) as tc:
        with (
            tc.tile_pool(name="io", bufs=3) as io_pool,
            tc.tile_pool(name="work", bufs=2) as work_pool,
            tc.tile_pool(name="persist", bufs=1) as persist,
            tc.tile_pool(name="dram", bufs=1, space="DRAM") as dram_pool,
        ):
            dbuf = persist.tile([P, free], f32, name="dbuf")
            sumd_cols = persist.tile([P, T], f32, name="sumd_cols")
            sumsq_cols = persist.tile([P, T], f32, name="sumsq_cols")
            erf_cols = persist.tile([P, TE], f32, name="erf_cols")

            # Dummy activation: forces the single ACT table set containing
            # Abs+Square+Erf to load once, up front, off the critical path.
            dummy = persist.tile([1, 1], f32, name="dummy")
            zca = nc.const_aps.tensor(0.0, (1, 1), f32)
            nc.scalar.activation(dummy[0:1, 0:1], zca, act_fn)

            # Phase A: stream tiles; d = |pred-target| stays in SBUF.
            # ACT accumulates per-partition sum|d| and (in-place) sum d^2.
            for t in range(T):
                sl = slice(t * f_tile, (t + 1) * f_tile)
                pt = io_pool.tile([P, f_tile], f32, name="pt", tag="pt")
                tt = io_pool.tile([P, f_tile], f32, name="tt", tag="tt")
                nc.sync.dma_start(out=pt[:], in_=pred[:, sl])
                nc.sync.dma_start(out=tt[:], in_=targ[:, sl])
                df = work_pool.tile([P, f_tile], f32, name="df", tag="df")
                nc.vector.tensor_sub(df[:], pt[:], tt[:])
                nc.scalar.activation(
                    dbuf[:, sl], df[:], AF.Abs,
                    accum_out=sumd_cols[:, t : t + 1],
                )
                nc.scalar.activation(
                    df[:], df[:], AF.Square,
                    accum_out=sumsq_cols[:, t : t + 1],
                )

            stats = persist.tile([P, 2], f32, name="stats")
            nc.vector.reduce_sum(stats[:, 0:1], sumd_cols[:], axis=X)
            nc.vector.reduce_sum(stats[:, 1:2], sumsq_cols[:], axis=X)

            # Per-core totals on every partition, then 8B-per-rank AllGather.
            totc = persist.tile([P, 2], f32, name="totc")
            nc.gpsimd.partition_all_reduce(
                totc[:], stats[:], channels=P, reduce_op=bass_isa.ReduceOp.add
            )
            cc_in = dram_pool.tile([1, 2], f32, name="cc_in")
            cc_out = dram_pool.tile([n_cores, 2], f32, name="cc_out",
                                    addr_space="Shared")
            nc.sync.dma_start(out=cc_in[:], in_=totc[0:1, :])
            nc.gpsimd.collective_compute(
                "AllGather", ALU.bypass,
                replica_groups=[list(range(n_cores))],
                ins=[cc_in.opt()], outs=[cc_out.opt()],
            )
            gt = persist.tile([1, 2 * n_cores], f32, name="gt")
            nc.sync.dma_start(out=gt[:], in_=cc_out[:])
            tot01 = persist.tile([1, 2], f32, name="tot01")
            nc.vector.reduce_sum(
                tot01[0:1, :],
                gt[:].rearrange("p (r j) -> p j r", r=n_cores),
                axis=X,
            )
            tot = persist.tile([P, 2], f32, name="tot")
            nc.gpsimd.partition_broadcast(tot[:], tot01[0:1, :])

            # var = (sum_sq - sum_d * mean) / (n-1);  s = INV_SQRT2 / var
            # computed redundantly on every partition ([P,1] lanes).
            mean = persist.tile([P, 1], f32, name="mean")
            nc.vector.tensor_scalar_mul(mean[:], tot[:, 0:1], 1.0 / n_total)
            t1 = persist.tile([P, 1], f32, name="t1")
            nc.vector.tensor_mul(t1[:], mean[:], tot[:, 0:1])
            t2 = persist.tile([P, 1], f32, name="t2")
            nc.vector.tensor_sub(t2[:], tot[:, 1:2], t1[:])
            var = persist.tile([P, 1], f32, name="var")
            nc.vector.tensor_scalar_mul(var[:], t2[:], 1.0 / (n_total - 1.0))
            rvar = persist.tile([P, 1], f32, name="rvar")
            nc.vector.reciprocal(rvar[:], var[:])
            svec = persist.tile([P, 1], f32, name="svec")
            nc.vector.tensor_scalar_mul(svec[:], rvar[:], INV_SQRT2)

            # Phase B: erf(s*d) in-place over the resident d buffer; d >= 0
            # so the ACT accumulator sum is already the sum of erf values.
            for t in range(TE):
                sl = slice(t * f_erf, (t + 1) * f_erf)
                nc.scalar.activation(
                    dbuf[:, sl], dbuf[:, sl], act_fn,
                    scale=svec[:, 0:1],
                    accum_out=erf_cols[:, t : t + 1],
                )

            outsb = persist.tile([P, 3], f32, name="outsb")
            nc.vector.reduce_sum(outsb[:, 0:1], erf_cols[:], axis=X)
            nc.vector.tensor_copy(outsb[:, 1:3], tot[:])
            nc.sync.dma_start(out=out[:, :], in_=outsb[:])

    nc.finalize()
    return nc


def _get_nc():
    if "nc" not in _CACHE:
        _CACHE["nc"] = _build()
    return _CACHE["nc"]


def _finish(results):
    """Host-side O(1) scalar math from per-core device sums."""
    r0 = np.asarray(results[0]["out"], dtype=np.float64)
    sum_d = r0[0, 1]
    sum_sq = r0[0, 2]
    s_erf = 0.0
    for r in results:
        s_erf += float(np.asarray(r["out"])[:, 0].sum(dtype=np.float64))
    n = float(N_TOTAL)
    mean_d = sum_d / n
    var = (sum_sq - sum_d * mean_d) / (n - 1.0)
    p = 1.0 - s_erf / n
    gamma = -np.log(p)
    loss = mean_d * (1.0 - p) ** gamma + np.log1p(var)
    return np.array(loss, dtype=np.float32)


def kernel(pred: np.ndarray, target: np.ndarray) -> np.ndarray:
    from concourse.bass_utils import run_bass_kernel_spmd

    nc = _get_nc()
    p = np.ascontiguousarray(pred, dtype=np.float32).reshape(-1)
    t = np.ascontiguousarray(target, dtype=np.float32).reshape(-1)
    in_maps = []
    for c in range(N_CORES):
        sl = slice(c * PER_CORE, (c + 1) * PER_CORE)
        in_maps.append({
            "pred": p[sl].reshape(P, FREE),
            "target": t[sl].reshape(P, FREE),
        })
    res = run_bass_kernel_spmd(nc, in_maps, list(range(N_CORES)))
    return _finish(res.results)
